# revision 25
# baseline (speedup 1.0000x reference)
"""Trainium2 Bass kernel for nn_Difference (ignorematch mode).

Math: result[i,j] = sum_k a_fk[i,k] * (a_fk[i,k] > 0) * (b_fk[j,k] <= 0)
where a_fk = a @ feats.T, b_fk = b @ feats.T.  This factorizes into three
matmuls with elementwise ops between them:

    P = relu(a @ feats.T)            # [Na, K]
    Q = (b @ feats.T) <= 0           # [Nb, K], exactly {0.0, 1.0}
    result = P @ Q.T                 # [Na, Nb]

No [Na, Nb, K] tensor is ever materialized.

Sharding: 4x2 grid over the output. Core (r, q) computes
result[r*256:(r+1)*256, q*512:(q+1)*512] from a-quarter r and b-half q;
feats is replicated.  JB=512 makes the b-side and final matmuls full
512-column (one PSUM bank) passes, so LDWEIGHTS is always hidden.

Precision: b-side runs in fp16 (fp8 b/feats flips ~2.7k of 262k mask
bits -> 2.3e-2 norm rel err, OVER the 2e-2 gate -- measured on the fixed
test inputs).  a ships as fp8-e4m3 raw bytes packed into the fp16 ring
buffer (bitcast view on device, DVE upcasts to fp16 before the a-side
matmuls); a-side matmuls/relu/finals then run fp16.  Measured norm rel
err with fp8-a + fp16 rest: 5.7e-3 vs the 2e-2 gate.

DMA plan.  Only the two HWDGE rings (Sync/Scalar) are usable (SWDGE
measured 7us+ for 256KB under load).  One packed input DMA per ring
(448KB each, was 512KB before the fp8-a cut); fewer DMAs per ring =
fewer serialized completion receipts.  Per ring h: d-chunks 2h, 2h+1 of
[feats.T | b.T-half] as fp16 plus both a.T-quarter chunks as packed fp8
bytes.  Outputs leave as two 128KB fp16 pieces, one per ring; host
upcasts to f32 during the unshard.

PE warmup: the HAM clock gate keeps the PE at ~1.2 GHz until it has been
busy-without-gaps for ~5.5us; any PE idle gap delays the 2.4 GHz ramp.
Dummy matmuls bridge from preamble end until the input semaphores land
on every core (see N_WARM), and the ring-phased main loop keeps the PE
gap-free from there.
"""

import os
import sys

import numpy as np
import ml_dtypes

sys.path.insert(0, "/opt/trn_rl_repo")

import concourse.bacc as bacc  # noqa: E402
import concourse.tile as tile  # noqa: E402
from concourse import mybir  # noqa: E402
from concourse.bass_utils import run_bass_kernel_spmd  # noqa: E402

# Problem shapes (hardcoded per contract).
NA, NB, D, K = 1024, 1024, 512, 256
A_SPLIT, B_SPLIT = 4, 2  # 8 cores in a 4x2 grid over the output
IA = NA // A_SPLIT  # 256 output rows per core
JB = NB // B_SPLIT  # 512 output cols per core
P = 128
DC = D // P  # 4 contraction chunks
KC = K // P  # 2 feature-bank chunks
MC = IA // P  # 2 output row chunks
FB = K + JB  # packed feats+b row length per (partition, dc): 768
AS = IA // 2  # fp16 slots holding one d-chunk of a.T fp8 bytes: 128
HW = 2 * FB + 2 * AS  # one ring's packed row: fb rows then fp8 a bytes (1792)

F32 = mybir.dt.float32
F16 = mybir.dt.float16
F8 = mybir.dt.float8e4

# Dummy 256-col matmuls bridging from engine-preamble end until the input
# DMAs land.  Sized for the worst-case (sem - warmup_start) across cores:
# late-preamble cores see their sem ~3.4us after warm start, early cores
# ~4.7us (HBM contention tail).  A PE gap costs ~1.5us (idle + delayed
# HAM clock ramp); overshoot costs only the overshoot.  Note the HAM ramp
# needs ~5.5us of gap-free PE before 2.4GHz: real matmuls issued before
# that point run at 1.2GHz, which is work-rate-neutral vs more warmup, so
# N_WARM is sized to the bridge (23x213ns = 4.9us), not the ramp.
N_WARM = 22

_BUILT = None
LAST_RESULTS = None


def _build():
    nc = bacc.Bacc(
        "TRN2", target_bir_lowering=False, debug=False, monotonic_sem_count=0
    )

    # Per-ring packed inputs, one contiguous 3.5KB run per partition:
    # inH[p, 0:FB]        = [feats.T ; b.T half] rows for d-chunk 2H
    # inH[p, FB:2FB]      = same for d-chunk 2H+1
    # inH[p, 2FB:2FB+AS]  = a.T quarter rows for d-chunk 2H, fp8 bytes
    #                       packed 2-per-fp16-slot
    # inH[p, 2FB+AS:]     = same for d-chunk 2H+1
    in0 = nc.dram_tensor("in0", [P, HW], F16, kind="ExternalInput")
    in1 = nc.dram_tensor("in1", [P, HW], F16, kind="ExternalInput")
    out = nc.dram_tensor("out", [P, MC, 2, JB // 2], F16, kind="ExternalOutput")

    with tile.TileContext(nc) as tc:
        with (
            tc.tile_pool(name="ins", bufs=1) as in_pool,
            tc.tile_pool(name="mid", bufs=1) as mid_pool,
            tc.tile_pool(name="outs", bufs=1) as out_pool,
            tc.tile_pool(name="ps_b", bufs=2, space="PSUM") as ps_b_pool,
            tc.tile_pool(name="ps_a", bufs=2, space="PSUM") as ps_a_pool,
            tc.tile_pool(name="ps_o", bufs=4, space="PSUM") as ps_o_pool,
        ):
            # Two DMAs per HWDGE ring: the 384KB feats+b block first (it
            # gates the bridge and all b-side matmuls), the 64KB packed
            # fp8 a block right behind it.  Receipts pipeline (~0.4us
            # measured, not serialized 2.4us), so the fb semaphore lands
            # ~0.35us earlier than with one 448KB DMA, and the a8 sem
            # ~0.5us later -- still well before the a-side matmuls (which
            # sit behind the four b-side ones).  Separate tiles per
            # section: dep tracking is tile-granular, so a single tile
            # would make the b-matmuls wait on the a8 DMA too.
            fb_sb = [
                in_pool.tile([P, 2 * FB], F16, tag=f"fb{h}", name=f"fb_sb{h}")
                for h in range(2)
            ]
            a8_sb = [
                in_pool.tile([P, 2 * AS], F16, tag=f"a8{h}", name=f"a8_sb{h}")
                for h in range(2)
            ]
            ins = (in0, in1)
            for h in range(2):
                eng = nc.sync if h == 0 else nc.scalar
                eng.dma_start(out=fb_sb[h][:], in_=ins[h][:, 0 : 2 * FB])
                eng.dma_start(out=a8_sb[h][:], in_=ins[h][:, 2 * FB : HW])

            # PE clock warmup while the DMAs fly.  A half-width warm tile
            # keeps the memset short (the framework requires the tile be
            # written before the PE reads it), so the first warmup matmul
            # issues as early as Vector clears its preamble.
            # memset on GpSimd: it clears its preamble earliest and keeps
            # DVE free, so the first warmup matmul issues sooner (an
            # earlier warm start also ramps the PE clock earlier).
            warm_sb = in_pool.tile([P, 256], F16, tag="warm", name="warm_sb")
            nc.gpsimd.memset(warm_sb[:], 0.0)

            def fT(dc, kc):  # feats.T chunk [128d, 128k]
                return fb_sb[dc // 2][:, (dc % 2) * FB + kc * P :][:, 0:P]

            def bT(dc):  # b.T chunk [128d, 512j]
                return fb_sb[dc // 2][:, (dc % 2) * FB + K :][:, 0:JB]

            # Upcast the packed fp8 a.T bytes to fp16 working tiles, one
            # contiguous DVE copy per ring (both d-chunks at once).  Runs
            # as soon as that ring's input sem lands, well before the
            # a-side matmuls need it.  (fp8 DoubleRow matmuls were tried
            # instead: the ISA rejects DR outputs at PSUM partition
            # offset 64, DR is only ~1.44x on HW and disables FWL -- a
            # net loss here.)
            # Four per-d-chunk copies (not one per ring): aT(0) is ready
            # ~0.2us after the a8 sem, just in time for the first a-side
            # matmul behind the four b-side ones (a whole-ring upcast
            # left the PE stalling ~0.3us twice per core).
            # Separate tiles per d-chunk: dep tracking is tile-granular,
            # so a shared tile would make aT(0) readers wait on all four
            # upcasts.
            aT16 = [
                mid_pool.tile([P, IA], F16, tag=f"a16_{dc}", name=f"aT16_{dc}")
                for dc in range(DC)
            ]
            for dc in range(DC):
                nc.vector.tensor_copy(
                    aT16[dc][:],
                    a8_sb[dc // 2][:, (dc % 2) * AS : (dc % 2 + 1) * AS].bitcast(F8),
                )

            def aT(dc):  # a.T chunk [128d, 256i], upcast from fp8
                return aT16[dc][:]

            # Per-kc tiles (not one [P, KC, ...] tile) so a final matmul
            # reading kc=0 doesn't pick up a false dep on the kc=1 evict.
            QT_sb = [
                mid_pool.tile([P, JB], F16, tag=f"qt{kc}", name=f"QT{kc}")
                for kc in range(KC)
            ]
            PT_sb = [
                mid_pool.tile([P, IA], F16, tag=f"pt{kc}", name=f"PT{kc}")
                for kc in range(KC)
            ]
            # Output staging: four [P, 256] quarters (mc, jh), evicted and
            # DMA'd as each closes so the out rings spin up early.
            out_sb = out_pool.tile([P, MC, 2, JB // 2], F16, tag="osb")

            ps_b = [
                ps_b_pool.tile([P, JB], F32, tag="psb", name=f"ps_b{kc}")
                for kc in range(KC)
            ]
            ps_a = [
                ps_a_pool.tile([P, IA], F32, tag="psa", name=f"ps_a{kc}")
                for kc in range(KC)
            ]
            ps_o = [
                ps_o_pool.tile([P, JB // 2], F32, tag="pso", name=f"ps_o{qq}")
                for qq in range(4)
            ]

            # PE clock warmup while the DMAs fly; targets ps_o[0], which
            # the finals don't touch until long after the warm stream
            # drains (same engine, in-order), so no extra bank is needed.
            for _ in range(N_WARM):
                nc.tensor.matmul(
                    ps_o[0][:],
                    lhsT=warm_sb[:, 0:P],
                    rhs=warm_sb[:],
                    start=True,
                    stop=True,
                )

            # Ring-phased accumulation: ALL of ring0's d-chunks (b-side and
            # a-side partial sums, ~2.5us of matmuls) run before anything
            # from ring1, so a late second DMA cannot stall the PE
            # mid-stream (per-core HBM arbitration makes one ring late on
            # some core most runs, and a PE gap also delays the HAM clock
            # ramp).  PSUM accumulation groups per bank pause across the
            # interleave (start on d0, stop on d3).
            # All four b-side matmuls run before any a-side one: the
            # a-side rhs comes from the DVE upcast (done ~0.45us after the
            # ring0 sem), so an a-matmul issued 3rd would stall the PE
            # right after the bridge (observed w=450ns waits) -- and a PE
            # gap there also delays the HAM clock ramp.
            for dc in (0, 1):  # ring0 phase, b-side
                for kc in range(KC):
                    nc.tensor.matmul(
                        ps_b[kc][:],
                        lhsT=fT(dc, kc),
                        rhs=bT(dc),
                        start=(dc == 0),
                        stop=False,
                    )
            for dc in (0, 1):  # ring0 phase, a-side
                for kc in range(KC):
                    nc.tensor.matmul(
                        ps_a[kc][:],
                        lhsT=fT(dc, kc),
                        rhs=aT(dc),
                        start=(dc == 0),
                        stop=False,
                    )
            # ring1 phase, interleaved closes b0, a0, b1, a1: the kc=0
            # ingredients (QT0 via is_le on DVE, PT0 via relu on ACT, in
            # parallel) are ready ~0.4us earlier than with b0,b1,a0,a1,
            # so the kc=0 final pass starts right as the last a-close
            # drains.  relu on ACT: the act-table load it hoists onto the
            # Scalar queue delays the in1 DMA ~1.3us -- absorbed by the
            # ring-phased schedule.
            def close_b(kc):
                for dc in (2, 3):
                    nc.tensor.matmul(
                        ps_b[kc][:],
                        lhsT=fT(dc, kc),
                        rhs=bT(dc),
                        start=False,
                        stop=(dc == 3),
                    )
                nc.vector.tensor_scalar(
                    QT_sb[kc][:], ps_b[kc][:], 0.0, None, mybir.AluOpType.is_le
                )

            def close_a(kc):
                for dc in (2, 3):
                    nc.tensor.matmul(
                        ps_a[kc][:],
                        lhsT=fT(dc, kc),
                        rhs=aT(dc),
                        start=False,
                        stop=(dc == 3),
                    )
                nc.scalar.activation(
                    PT_sb[kc][:], ps_a[kc][:], mybir.ActivationFunctionType.Relu
                )

            # (A sigmoid(-1e6*x) step mask on ACT was tried for QT1 to
            # break the serial is_le chain on DVE: numerically identical,
            # but sigmoid lives in a different ACT table set than
            # relu/copy, and the mid-kernel ACT_TABLE_LOAD it forces
            # costs ~1.3us right in the tail -- net loss.)
            close_b(0)
            close_a(0)
            close_b(1)
            close_a(1)

            # Finals: out[i,j] = sum_k PT[k,i] * QT[k,j], as four [128,256]
            # quarters qq=(mc, jh).  One full kc=0 pass then the kc=1 pass,
            # so quarters close back-to-back and each evict+DMA overlaps
            # the rest: the out rings pay their ~1.4us spin-up while the
            # last quarters still compute, and only the last 64KB piece's
            # data+receipt sits on the critical tail.
            for kc in range(KC):
                for qq in range(4):
                    mc, jh = divmod(qq, 2)
                    nc.tensor.matmul(
                        ps_o[qq][:],
                        lhsT=PT_sb[kc][:, mc * P : (mc + 1) * P],
                        rhs=QT_sb[kc][:, jh * 256 : (jh + 1) * 256],
                        start=(kc == 0),
                        stop=(kc == KC - 1),
                    )

            # Evict quarters as they close (cast f32->fp16): DVE takes
            # qq=0,2 -> Sync ring, ACT takes qq=1,3 -> Scalar ring.
            def evict(qq):
                mc, jh = divmod(qq, 2)
                dst = out_sb[:, mc, jh, :]
                if qq % 2 == 0:
                    nc.vector.tensor_copy(dst, ps_o[qq][:])
                    nc.sync.dma_start(out=out[:, mc, jh, :], in_=dst)
                else:
                    nc.scalar.activation(
                        dst, ps_o[qq][:], mybir.ActivationFunctionType.Copy
                    )
                    nc.scalar.dma_start(out=out[:, mc, jh, :], in_=dst)

            for qq in range(4):
                evict(qq)

    nc.finalize()
    return nc


def kernel(a, b, feats):
    global _BUILT, LAST_RESULTS
    a = np.ascontiguousarray(a, dtype=np.float32)
    b = np.ascontiguousarray(b, dtype=np.float32)
    feats = np.ascontiguousarray(feats, dtype=np.float32)

    if _BUILT is None:
        _BUILT = _build()
    nc = _BUILT

    fT_r = np.ascontiguousarray(feats.T).astype(np.float16).reshape(DC, P, K)
    bT_r = np.ascontiguousarray(b.T).astype(np.float16).reshape(DC, P, NB)
    # a quantizes fp32 -> fp8-e4m3 on the host; the device upcast to fp16
    # is exact.  Raw bytes ride in the fp16 ring buffer (2 per slot).
    a8_r = (
        np.ascontiguousarray(a.T).astype(ml_dtypes.float8_e4m3fn).reshape(DC, P, NA)
    )

    in_maps = []
    for r in range(A_SPLIT):
        for q in range(B_SPLIT):
            ins = {}
            for h in range(2):
                buf = np.empty((P, HW), dtype=np.float16)
                for j in range(2):
                    dc = 2 * h + j
                    buf[:, j * FB : j * FB + K] = fT_r[dc]
                    buf[:, j * FB + K : (j + 1) * FB] = bT_r[
                        dc, :, q * JB : (q + 1) * JB
                    ]
                    a8 = np.ascontiguousarray(
                        a8_r[dc, :, r * IA : (r + 1) * IA]
                    )  # [P, IA] fp8
                    buf[:, 2 * FB + j * AS : 2 * FB + (j + 1) * AS] = (
                        a8.view(np.uint8).view(np.float16)
                    )
                ins[f"in{h}"] = buf
            in_maps.append(ins)

    kwargs = {}
    if os.environ.get("KERNEL_TRACE"):
        try:
            import antenv.axon_hooks  # noqa: F401  (shimmed by test.py)

            kwargs = dict(trace=True, trace_cores=list(range(8)))
        except ImportError:
            pass
    res = run_bass_kernel_spmd(nc, in_maps, core_ids=list(range(8)), **kwargs)
    LAST_RESULTS = res

    out = np.empty((NA, NB), dtype=np.float32)
    for c, r_map in enumerate(res.results):
        r, q = divmod(c, B_SPLIT)
        # device out: [P, MC, 2, JB//2]; result rows are mc*128 + p,
        # cols are jh*256 + j
        tile_out = (
            r_map["out"].transpose(1, 0, 2, 3).reshape(IA, JB).astype(np.float32)
        )
        out[r * IA : (r + 1) * IA, q * JB : (q + 1) * JB] = tile_out
    return out


# revision 27
# speedup vs baseline: 1.0617x; 1.0617x over previous
"""Trainium2 Bass kernel for nn_Difference (ignorematch mode).

Math: result[i,j] = sum_k a_fk[i,k] * (a_fk[i,k] > 0) * (b_fk[j,k] <= 0)
where a_fk = a @ feats.T, b_fk = b @ feats.T.  This factorizes into three
matmuls with elementwise ops between them:

    P = relu(a @ feats.T)            # [Na, K]
    Q = (b @ feats.T) <= 0           # [Nb, K], exactly {0.0, 1.0}
    result = P @ Q.T                 # [Na, Nb]

No [Na, Nb, K] tensor is ever materialized.

Sharding: 4x2 grid over the output. Core (r, q) computes
result[r*256:(r+1)*256, q*512:(q+1)*512] from a-quarter r and b-half q;
feats is replicated.  JB=512 makes the b-side and final matmuls full
512-column (one PSUM bank) passes, so LDWEIGHTS is always hidden.

Precision: b-side runs in fp16 (fp8 b/feats flips ~2.7k of 262k mask
bits -> 2.3e-2 norm rel err, OVER the 2e-2 gate -- measured on the fixed
test inputs).  a ships as fp8-e4m3 raw bytes packed into the fp16 ring
buffer (bitcast view on device, DVE upcasts to fp16 before the a-side
matmuls); a-side matmuls/relu/finals then run fp16.  Measured norm rel
err with fp8-a + fp16 rest: 5.7e-3 vs the 2e-2 gate.

DMA plan.  Only the two HWDGE rings (Sync/Scalar) are usable (SWDGE
measured 7us+ for 256KB under load).  One packed input DMA per ring
(448KB each, was 512KB before the fp8-a cut); fewer DMAs per ring =
fewer serialized completion receipts.  Per ring h: d-chunks 2h, 2h+1 of
[feats.T | b.T-half] as fp16 plus both a.T-quarter chunks as packed fp8
bytes.  Outputs leave as two 128KB fp16 pieces, one per ring; host
upcasts to f32 during the unshard.

PE warmup: the HAM clock gate keeps the PE at ~1.2 GHz until it has been
busy-without-gaps for ~5.5us; any PE idle gap delays the 2.4 GHz ramp.
Dummy matmuls bridge from preamble end until the input semaphores land
on every core (see N_WARM), and the ring-phased main loop keeps the PE
gap-free from there.
"""

import os
import sys

import numpy as np
import ml_dtypes

sys.path.insert(0, "/opt/trn_rl_repo")

import concourse.bacc as bacc  # noqa: E402
import concourse.tile as tile  # noqa: E402
from concourse import mybir  # noqa: E402
from concourse.bass_utils import run_bass_kernel_spmd  # noqa: E402

# Problem shapes (hardcoded per contract).
NA, NB, D, K = 1024, 1024, 512, 256
A_SPLIT, B_SPLIT = 4, 2  # 8 cores in a 4x2 grid over the output
IA = NA // A_SPLIT  # 256 output rows per core
JB = NB // B_SPLIT  # 512 output cols per core
P = 128
DC = D // P  # 4 contraction chunks
KC = K // P  # 2 feature-bank chunks
MC = IA // P  # 2 output row chunks
FB = K + JB  # packed feats+b row length per (partition, dc): 768
AS = IA // 2  # fp16 slots holding one d-chunk of a.T fp8 bytes: 128
HW = 2 * FB + 2 * AS  # one ring's packed row: fb rows then fp8 a bytes (1792)

F32 = mybir.dt.float32
F16 = mybir.dt.float16
F8 = mybir.dt.float8e4

# Dummy 256-col matmuls bridging from engine-preamble end until the input
# DMAs land.  Sized for the worst-case (sem - warmup_start) across cores:
# late-preamble cores see their sem ~3.4us after warm start, early cores
# ~4.7us (HBM contention tail).  A PE gap costs ~1.5us (idle + delayed
# HAM clock ramp); overshoot costs only the overshoot.  Note the HAM ramp
# needs ~5.5us of gap-free PE before 2.4GHz: real matmuls issued before
# that point run at 1.2GHz, which is work-rate-neutral vs more warmup, so
# N_WARM is sized to the bridge (23x213ns = 4.9us), not the ramp.
N_WARM = 23

_BUILT = None
LAST_RESULTS = None


def _build():
    nc = bacc.Bacc(
        "TRN2", target_bir_lowering=False, debug=False, monotonic_sem_count=0
    )

    # Per-ring packed inputs, one contiguous 3.5KB run per partition:
    # inH[p, 0:FB]        = [feats.T ; b.T half] rows for d-chunk 2H
    # inH[p, FB:2FB]      = same for d-chunk 2H+1
    # inH[p, 2FB:2FB+AS]  = a.T quarter rows for d-chunk 2H, fp8 bytes
    #                       packed 2-per-fp16-slot
    # inH[p, 2FB+AS:]     = same for d-chunk 2H+1
    in0 = nc.dram_tensor("in0", [P, HW], F16, kind="ExternalInput")
    in1 = nc.dram_tensor("in1", [P, HW], F16, kind="ExternalInput")
    out = nc.dram_tensor("out", [P, MC, 2, JB // 2], F16, kind="ExternalOutput")

    with tile.TileContext(nc) as tc:
        with (
            tc.tile_pool(name="ins", bufs=1) as in_pool,
            tc.tile_pool(name="mid", bufs=1) as mid_pool,
            tc.tile_pool(name="outs", bufs=1) as out_pool,
            tc.tile_pool(name="ps_b", bufs=2, space="PSUM") as ps_b_pool,
            tc.tile_pool(name="ps_a", bufs=2, space="PSUM") as ps_a_pool,
            tc.tile_pool(name="ps_o", bufs=4, space="PSUM") as ps_o_pool,
        ):
            # One DMA per HWDGE ring, first instruction on each engine.
            # (Splitting each ring into feats+b then a8 DMAs was tried:
            # the fb semaphore does land ~0.35us earlier, but the second
            # DMA's completion receipt serializes behind the first's and
            # lands ~1.5us after its data, stalling the first a-side
            # matmul on every core -- net loss.)
            in_sb = [
                in_pool.tile([P, HW], F16, tag=f"in{h}", name=f"in_sb{h}")
                for h in range(2)
            ]
            nc.sync.dma_start(out=in_sb[0][:], in_=in0[:])
            nc.scalar.dma_start(out=in_sb[1][:], in_=in1[:])

            # PE clock warmup while the DMAs fly.  A half-width warm tile
            # keeps the memset short (the framework requires the tile be
            # written before the PE reads it), so the first warmup matmul
            # issues as early as Vector clears its preamble.
            # memset on GpSimd: it clears its preamble earliest and keeps
            # DVE free, so the first warmup matmul issues sooner (an
            # earlier warm start also ramps the PE clock earlier).
            warm_sb = in_pool.tile([P, 256], F16, tag="warm", name="warm_sb")
            nc.gpsimd.memset(warm_sb[:], 0.0)

            def fT(dc, kc):  # feats.T chunk [128d, 128k]
                return in_sb[dc // 2][:, (dc % 2) * FB + kc * P :][:, 0:P]

            def bT(dc):  # b.T chunk [128d, 512j]
                return in_sb[dc // 2][:, (dc % 2) * FB + K :][:, 0:JB]

            # Upcast the packed fp8 a.T bytes to fp16 working tiles, one
            # contiguous DVE copy per ring (both d-chunks at once).  Runs
            # as soon as that ring's input sem lands, well before the
            # a-side matmuls need it.  (fp8 DoubleRow matmuls were tried
            # instead: the ISA rejects DR outputs at PSUM partition
            # offset 64, DR is only ~1.44x on HW and disables FWL -- a
            # net loss here.)
            aT16 = [
                mid_pool.tile([P, 2 * IA], F16, tag=f"a16_{h}", name=f"aT16_{h}")
                for h in range(2)
            ]
            nc.vector.tensor_copy(
                aT16[0][:], in_sb[0][:, 2 * FB : 2 * FB + 2 * AS].bitcast(F8)
            )
            nc.vector.tensor_copy(
                aT16[1][:], in_sb[1][:, 2 * FB : 2 * FB + 2 * AS].bitcast(F8)
            )

            def aT(dc):  # a.T chunk [128d, 256i], upcast from fp8
                return aT16[dc // 2][:, (dc % 2) * IA :][:, 0:IA]

            # Per-kc tiles (not one [P, KC, ...] tile) so a final matmul
            # reading kc=0 doesn't pick up a false dep on the kc=1 evict.
            QT_sb = [
                mid_pool.tile([P, JB], F16, tag=f"qt{kc}", name=f"QT{kc}")
                for kc in range(KC)
            ]
            PT_sb = [
                mid_pool.tile([P, IA], F16, tag=f"pt{kc}", name=f"PT{kc}")
                for kc in range(KC)
            ]
            # Output staging: four [P, 256] quarters (mc, jh), evicted and
            # DMA'd as each closes so the out rings spin up early.
            out_sb = out_pool.tile([P, MC, 2, JB // 2], F16, tag="osb")

            ps_b = [
                ps_b_pool.tile([P, JB], F32, tag="psb", name=f"ps_b{kc}")
                for kc in range(KC)
            ]
            ps_a = [
                ps_a_pool.tile([P, IA], F32, tag="psa", name=f"ps_a{kc}")
                for kc in range(KC)
            ]
            ps_o = [
                ps_o_pool.tile([P, JB // 2], F32, tag="pso", name=f"ps_o{qq}")
                for qq in range(4)
            ]

            # PE clock warmup while the DMAs fly; targets ps_o[0], which
            # the finals don't touch until long after the warm stream
            # drains (same engine, in-order), so no extra bank is needed.
            for _ in range(N_WARM):
                nc.tensor.matmul(
                    ps_o[0][:],
                    lhsT=warm_sb[:, 0:P],
                    rhs=warm_sb[:],
                    start=True,
                    stop=True,
                )

            # Ring-phased accumulation: ALL of ring0's d-chunks (b-side and
            # a-side partial sums, ~2.5us of matmuls) run before anything
            # from ring1, so a late second DMA cannot stall the PE
            # mid-stream (per-core HBM arbitration makes one ring late on
            # some core most runs, and a PE gap also delays the HAM clock
            # ramp).  PSUM accumulation groups per bank pause across the
            # interleave (start on d0, stop on d3).
            # All four b-side matmuls run before any a-side one: the
            # a-side rhs comes from the DVE upcast (done ~0.45us after the
            # ring0 sem), so an a-matmul issued 3rd would stall the PE
            # right after the bridge (observed w=450ns waits) -- and a PE
            # gap there also delays the HAM clock ramp.
            for dc in (0, 1):  # ring0 phase, b-side
                for kc in range(KC):
                    nc.tensor.matmul(
                        ps_b[kc][:],
                        lhsT=fT(dc, kc),
                        rhs=bT(dc),
                        start=(dc == 0),
                        stop=False,
                    )
            for dc in (0, 1):  # ring0 phase, a-side
                for kc in range(KC):
                    nc.tensor.matmul(
                        ps_a[kc][:],
                        lhsT=fT(dc, kc),
                        rhs=aT(dc),
                        start=(dc == 0),
                        stop=False,
                    )
            # ring1 phase, interleaved closes b0, a0, b1, a1: the kc=0
            # ingredients (QT0 via is_le on DVE, PT0 via relu on ACT, in
            # parallel) are ready ~0.4us earlier than with b0,b1,a0,a1,
            # so the kc=0 final pass starts right as the last a-close
            # drains.  relu on ACT: the act-table load it hoists onto the
            # Scalar queue delays the in1 DMA ~1.3us -- absorbed by the
            # ring-phased schedule.
            def close_b(kc):
                for dc in (2, 3):
                    nc.tensor.matmul(
                        ps_b[kc][:],
                        lhsT=fT(dc, kc),
                        rhs=bT(dc),
                        start=False,
                        stop=(dc == 3),
                    )
                nc.vector.tensor_scalar(
                    QT_sb[kc][:], ps_b[kc][:], 0.0, None, mybir.AluOpType.is_le
                )

            def close_a(kc):
                for dc in (2, 3):
                    nc.tensor.matmul(
                        ps_a[kc][:],
                        lhsT=fT(dc, kc),
                        rhs=aT(dc),
                        start=False,
                        stop=(dc == 3),
                    )
                nc.scalar.activation(
                    PT_sb[kc][:], ps_a[kc][:], mybir.ActivationFunctionType.Relu
                )

            # (A sigmoid(-1e6*x) step mask on ACT was tried for QT1 to
            # break the serial is_le chain on DVE: numerically identical,
            # but sigmoid lives in a different ACT table set than
            # relu/copy, and the mid-kernel ACT_TABLE_LOAD it forces
            # costs ~1.3us right in the tail -- net loss.)
            close_b(0)
            close_a(0)
            close_b(1)
            close_a(1)

            # Finals: out[i,j] = sum_k PT[k,i] * QT[k,j], as four [128,256]
            # quarters qq=(mc, jh).  One full kc=0 pass then the kc=1 pass,
            # so quarters close back-to-back and each evict+DMA overlaps
            # the rest: the out rings pay their ~1.4us spin-up while the
            # last quarters still compute, and only the last 64KB piece's
            # data+receipt sits on the critical tail.
            for kc in range(KC):
                for qq in range(4):
                    mc, jh = divmod(qq, 2)
                    nc.tensor.matmul(
                        ps_o[qq][:],
                        lhsT=PT_sb[kc][:, mc * P : (mc + 1) * P],
                        rhs=QT_sb[kc][:, jh * 256 : (jh + 1) * 256],
                        start=(kc == 0),
                        stop=(kc == KC - 1),
                    )

            # Evict quarters as they close (cast f32->fp16): DVE takes
            # qq=0,2 -> Sync ring, ACT takes qq=1,3 -> Scalar ring.
            def evict(qq):
                mc, jh = divmod(qq, 2)
                dst = out_sb[:, mc, jh, :]
                if qq % 2 == 0:
                    nc.vector.tensor_copy(dst, ps_o[qq][:])
                    nc.sync.dma_start(out=out[:, mc, jh, :], in_=dst)
                else:
                    nc.scalar.activation(
                        dst, ps_o[qq][:], mybir.ActivationFunctionType.Copy
                    )
                    nc.scalar.dma_start(out=out[:, mc, jh, :], in_=dst)

            for qq in range(4):
                evict(qq)

    nc.finalize()
    return nc


def kernel(a, b, feats):
    global _BUILT, LAST_RESULTS
    a = np.ascontiguousarray(a, dtype=np.float32)
    b = np.ascontiguousarray(b, dtype=np.float32)
    feats = np.ascontiguousarray(feats, dtype=np.float32)

    if _BUILT is None:
        _BUILT = _build()
    nc = _BUILT

    fT_r = np.ascontiguousarray(feats.T).astype(np.float16).reshape(DC, P, K)
    bT_r = np.ascontiguousarray(b.T).astype(np.float16).reshape(DC, P, NB)
    # a quantizes fp32 -> fp8-e4m3 on the host; the device upcast to fp16
    # is exact.  Raw bytes ride in the fp16 ring buffer (2 per slot).
    a8_r = (
        np.ascontiguousarray(a.T).astype(ml_dtypes.float8_e4m3fn).reshape(DC, P, NA)
    )

    in_maps = []
    for r in range(A_SPLIT):
        for q in range(B_SPLIT):
            ins = {}
            for h in range(2):
                buf = np.empty((P, HW), dtype=np.float16)
                for j in range(2):
                    dc = 2 * h + j
                    buf[:, j * FB : j * FB + K] = fT_r[dc]
                    buf[:, j * FB + K : (j + 1) * FB] = bT_r[
                        dc, :, q * JB : (q + 1) * JB
                    ]
                    a8 = np.ascontiguousarray(
                        a8_r[dc, :, r * IA : (r + 1) * IA]
                    )  # [P, IA] fp8
                    buf[:, 2 * FB + j * AS : 2 * FB + (j + 1) * AS] = (
                        a8.view(np.uint8).view(np.float16)
                    )
                ins[f"in{h}"] = buf
            in_maps.append(ins)

    kwargs = {}
    if os.environ.get("KERNEL_TRACE"):
        try:
            import antenv.axon_hooks  # noqa: F401  (shimmed by test.py)

            kwargs = dict(trace=True, trace_cores=list(range(8)))
        except ImportError:
            pass
    res = run_bass_kernel_spmd(nc, in_maps, core_ids=list(range(8)), **kwargs)
    LAST_RESULTS = res

    out = np.empty((NA, NB), dtype=np.float32)
    for c, r_map in enumerate(res.results):
        r, q = divmod(c, B_SPLIT)
        # device out: [P, MC, 2, JB//2]; result rows are mc*128 + p,
        # cols are jh*256 + j
        tile_out = (
            r_map["out"].transpose(1, 0, 2, 3).reshape(IA, JB).astype(np.float32)
        )
        out[r * IA : (r + 1) * IA, q * JB : (q + 1) * JB] = tile_out
    return out
